# revision 18
# baseline (speedup 1.0000x reference)
"""Trainium2 Bass kernel for AlphaFold-style gated MSA attention.

Reference computation (per batch b=1, per MSA row n of 64):
    q = (q_x @ wq) / sqrt(32);  k = k_x @ wk;  v = v_x @ wv      (heads: 8 x 32)
    a = softmax(q k^T + bias_mask[n,k] + bias_pair[h,q,k])
    o = (a @ v) * sigmoid(q_x @ wg + bg)
    out = o @ wo + bo

Distribution: data-parallel over the 64 MSA rows -> 8 rows per NeuronCore.

Design (v2, bf16):
  * All matmul operands bf16 (1 cyc/row on PE, same as f32r, but halves
    DMA/SBUF traffic and speeds weight loads). Logits/accumulations stay
    f32 in PSUM. Measured end-to-end rel err ~6e-3 vs the 2e-2 gate.
  * S^T layout ([k, q]) so bias_mask folds into the ACT exp as a
    per-partition bias. bias_pair is applied multiplicatively AFTER exp:
    host ships EBP = exp(bias_pair^T) in bf16; one DVE tensor_mul per
    head-pair runs at the 2x_1p rate (all-bf16), replacing the baseline's
    expensive PE-identity / GPSIMD bias paths.
  * exp runs on ACT in [128, 2, 512] chunks (2 heads x 1 key-chunk),
    writing bf16 E. No max-subtraction (logits are O(10), f32/bf16 safe).
  * AV packs 2 heads per PSUM bank (tile_position cols 0/64); the v tile
    carries a ones column so row 32/96 of each bank accumulates the
    softmax denominator for free.
  * The per-head [33, 512] outputs are staged to SBUF once per pair
    (GPSIMD copy), then repacked into outproj layout with 4 DMAs and the
    denominators broadcast with 8 tiny DMAs; 1/den via the fast DVE
    reciprocal approximation; gate = sigmoid via ACT tanh (same table as
    exp) + GPSIMD fixup.
  * Software pipelining: row n's tail (normalize/gate/outproj) is emitted
    during row n+1; AV of pair p is emitted during QK of pair p+1 so the
    PE never waits on the ACT exp pipeline.
"""

import math
import os
import sys

for _p in ("/opt/trn_rl_repo", "/root/.axon_site/_ro/trn_rl_repo"):
    if os.path.isdir(_p) and _p not in sys.path:
        sys.path.append(_p)

import ml_dtypes
import numpy as np

import bass_rust
import concourse.bass as bass
import concourse.mybir as mybir
import concourse.tile as tile
from concourse.bass_utils import run_bass_kernel_spmd
from concourse.tile import ScopedClock

f32 = mybir.dt.float32
bf16 = mybir.dt.bfloat16
BF = ml_dtypes.bfloat16

N_CORES = 8
NL = 8        # MSA rows per core (64 / 8)
SEQ = 512     # q and k sequence length
C = 256       # channel dim of q_x/k_x/v_x and the output
HID = 256     # heads * c_hidden
H = 8         # heads
CH = 32       # c_hidden per head
P = 128
CC = C // P   # 2 contraction chunks for projections
HC = HID // P  # 2 hidden chunks
KC = SEQ // P  # 4 key chunks
QC = SEQ // P  # 4 query chunks
HG = 2        # head groups of 4
PR = 4        # head pairs


class _TileContextSplitWaits(tile.TileContext):
    """This container's walrus supports ONE sync-wait per instruction (the
    TRN2 EVENTS struct has a single wait slot and this build refuses to
    expand multi-wait instructions). Tile attaches several waits to one
    instruction; split the extras onto same-engine NOPs emitted just before
    it — the engine queue is in-order, so this is semantically identical."""

    def _add_instruction(self, inst):
        si = inst.sync_info
        if (
            si is not None
            and len(si.on_wait) > 1
            and inst.engine != mybir.EngineType.Unassigned
        ):
            waits = list(si.on_wait)
            for w in waits[:-1]:
                nop = mybir.InstNoOp(
                    name=self.nc.get_next_instruction_name(),
                    sync_info=mybir.SyncInfo(on_wait=[w], on_update=[]),
                    bass_nofuse=True,
                    engine=inst.engine,
                )
                super()._add_instruction(nop)
            inst.sync_info = mybir.SyncInfo(
                on_wait=waits[-1:], on_update=list(si.on_update)
            )
        super()._add_instruction(inst)

    def _drain_and_barrier(self, tick_clock, wait_clock):
        nc = self.nc
        drain_inst = nc.sync.drain()
        wait_clock.add_sem_waits(
            drain_inst.ins, ScopedClock({None: tick_clock.global_clock})
        )
        si = drain_inst.ins.sync_info
        if si is not None and len(si.on_wait) > 1:
            waits = list(si.on_wait)
            updates = list(si.on_update)
            drain_inst.ins.sync_info = bass_rust.SyncInfo(
                on_wait=waits[:1], on_update=[]
            )
            for i, w in enumerate(waits[1:]):
                upd = updates if i == len(waits) - 2 else []
                nop = nc.sync.nop()
                nop.ins.sync_info = bass_rust.SyncInfo(on_wait=[w], on_update=upd)
        nc.all_engine_barrier()
        assert self.sems is not None
        popped = nc._tile_sem_poison_stack.pop()
        assert popped is self._sem_poison
        nc.clear_and_free_semaphores(list(self.sems.allocated().values()))
        nc.all_engine_barrier()


def _build_nc():
    nc = bass.Bass(
        "TRN2", target_bir_lowering=False, debug=False, num_devices=N_CORES
    )
    qx = nc.dram_tensor("qx", [NL, C, SEQ], bf16, kind="ExternalInput").ap()
    kx = nc.dram_tensor("kx", [NL, C, SEQ], bf16, kind="ExternalInput").ap()
    vx = nc.dram_tensor("vx", [NL, C, SEQ], bf16, kind="ExternalInput").ap()
    bpt = nc.dram_tensor(
        "bpt", [P, PR, KC, 2, SEQ], bf16, kind="ExternalInput"
    ).ap()
    bm = nc.dram_tensor("bm", [P, KC, NL], f32, kind="ExternalInput").ap()
    wq = nc.dram_tensor("wq", [C, HID], bf16, kind="ExternalInput").ap()
    wk = nc.dram_tensor("wk", [C, HID], bf16, kind="ExternalInput").ap()
    wv = nc.dram_tensor("wv", [C, HID], bf16, kind="ExternalInput").ap()
    wg = nc.dram_tensor("wg", [C, HID], bf16, kind="ExternalInput").ap()
    bgh = nc.dram_tensor("bgh", [P, HC], f32, kind="ExternalInput").ap()
    wo = nc.dram_tensor("wo", [HID, C], bf16, kind="ExternalInput").ap()
    bo_bc = nc.dram_tensor("bo_bc", [P, C], f32, kind="ExternalInput").ap()
    out = nc.dram_tensor("out", [NL, SEQ, C], f32, kind="ExternalOutput").ap()

    Exp = mybir.ActivationFunctionType.Exp
    Tanh = mybir.ActivationFunctionType.Tanh
    Copy = mybir.ActivationFunctionType.Copy
    MULT = mybir.AluOpType.mult
    ADD = mybir.AluOpType.add
    BYPASS = mybir.AluOpType.bypass

    with _TileContextSplitWaits(nc) as tc:
        with (
            tc.tile_pool(name="const", bufs=1) as const,
        ):
            # --- constants ---------------------------------------------------
            w_sbs = {}
            for name, w_ap in (("wq", wq), ("wk", wk), ("wv", wv), ("wg", wg)):
                w_sbs[name] = const.tile(
                    [P, CC, HID], bf16, tag=f"w_{name}", name=f"w_{name}"
                )
                nc.sync.dma_start(
                    out=w_sbs[name],
                    in_=w_ap.rearrange("(cc p) h -> p cc h", p=P),
                )
            wo_sb = const.tile([P, HC, C], bf16, tag="w_wo")
            nc.sync.dma_start(
                out=wo_sb, in_=wo.rearrange("(hc p) c -> p hc c", p=P)
            )
            bm_sb = const.tile([P, KC, NL], f32, tag="bm")
            nc.sync.dma_start(out=bm_sb, in_=bm)
            bgh_sb = const.tile([P, HC], f32, tag="bgh")
            nc.sync.dma_start(out=bgh_sb, in_=bgh)
            bo_sb = const.tile([P, C], f32, tag="bo")
            nc.sync.dma_start(out=bo_sb, in_=bo_bc)
            # exp(bias_pair^T), split per pair so pair 0 lands quickly
            bpt_sb = const.tile([P, PR, KC, 2, SEQ], bf16, tag="bpt")
            for pr in range(PR):
                nc.sync.dma_start(out=bpt_sb[:, pr], in_=bpt[:, pr])

            # --- main loop ---------------------------------------------------
            with (
                tc.tile_pool(name="xt", bufs=2) as xt,
                tc.tile_pool(name="drp", bufs=2, space="DRAM") as drp,
                tc.tile_pool(name="qk", bufs=2) as qk,
                tc.tile_pool(name="gh", bufs=2) as gh,
                tc.tile_pool(name="vv", bufs=2) as vv,
                tc.tile_pool(name="ee", bufs=2) as ee,
                tc.tile_pool(name="st", bufs=2) as st,
                tc.tile_pool(name="og", bufs=2) as og,
                tc.tile_pool(name="oo", bufs=2) as oo,
                tc.tile_pool(name="ou", bufs=2) as ou,
                tc.tile_pool(name="psP", bufs=2, space="PSUM") as psP,
                tc.tile_pool(name="psQ", bufs=2, space="PSUM") as psQ,
                tc.tile_pool(name="psO", bufs=2, space="PSUM") as psO,
            ):
                xts = {}

                def emit_dma_in(n):
                    t = {}
                    for name, src in (("q", qx), ("k", kx), ("v", vx)):
                        xT = xt.tile([P, CC, SEQ], bf16, tag=f"xt_{name}")
                        nc.sync.dma_start(
                            out=xT,
                            in_=src[n].rearrange("(cc p) s -> p cc s", p=P),
                        )
                        t[name] = xT
                    xts[n] = t

                def emit_proj(n):
                    xT = xts.pop(n)
                    qT = qk.tile([P, HG, SEQ], bf16, tag="qT")
                    kT = qk.tile([P, HG, SEQ], bf16, tag="kT")
                    for dst, wname, src in (
                        (qT, "wq", xT["q"]),
                        (kT, "wk", xT["k"]),
                    ):
                        for hc in range(HC):
                            pp = psP.tile([P, SEQ], f32, tag="psP")
                            for cc in range(CC):
                                nc.tensor.matmul(
                                    pp,
                                    w_sbs[wname][:, cc, P * hc : P * (hc + 1)],
                                    src[:, cc, :],
                                    start=(cc == 0),
                                    stop=(cc == CC - 1),
                                )
                            nc.vector.tensor_copy(dst[:, hc, :], pp)

                    gth = gh.tile([P, HC, SEQ], f32, tag="gth")
                    for hc in range(HC):
                        pp = psP.tile([P, SEQ], f32, tag="psP")
                        for cc in range(CC):
                            nc.tensor.matmul(
                                pp,
                                w_sbs["wg"][:, cc, P * hc : P * (hc + 1)],
                                xT["q"][:, cc, :],
                                start=(cc == 0),
                                stop=(cc == CC - 1),
                            )
                        # sigmoid(x + bg) = 0.5*tanh((x + bg)/2) + 0.5;
                        # the 0.5t+0.5 fixup runs on GPSIMD in the tail.
                        nc.scalar.activation(
                            gth[:, hc, :],
                            pp,
                            Tanh,
                            bias=bgh_sb[:, hc : hc + 1],
                            scale=0.5,
                        )

                    v_sb = vv.tile([P, KC, H, CH + 1], bf16, tag="v")
                    nc.gpsimd.memset(v_sb[:, :, :, CH : CH + 1], 1.0)
                    for rc in range(KC):
                        pp = psP.tile([P, SEQ], f32, tag="psP")
                        for cc in range(CC):
                            nc.tensor.matmul(
                                pp[:, 0:HID],
                                xT["v"][:, cc, P * rc : P * (rc + 1)],
                                w_sbs["wv"][:, cc, :],
                                start=(cc == 0),
                                stop=(cc == CC - 1),
                            )
                        # split the v evacuations between ACT (copy shares the
                        # exp table) and DVE to balance engine load
                        if rc < 2:
                            nc.scalar.activation(
                                v_sb[:, rc, :, 0:CH],
                                pp[:, 0:HID].rearrange("p (h c) -> p h c", h=H),
                                Copy,
                            )
                        else:
                            nc.vector.tensor_copy(
                                v_sb[:, rc, :, 0:CH],
                                pp[:, 0:HID].rearrange("p (h c) -> p h c", h=H),
                            )
                    return qT, kT, gth, v_sb

                def emit_pairs(n, qT, kT, gth, v_sb, tail_state):
                    stg = st.tile([P, PR, SEQ], f32, tag="stg")
                    oTg_raw = og.tile([P, HG, SEQ], f32, tag="oTg_raw")
                    rbc = og.tile([P, HG, SEQ], f32, tag="rbc")

                    def emit_av(item):
                        pr, Es = item
                        po = psO.tile([P, SEQ], f32, tag="po")
                        for j in range(2):
                            h = 2 * pr + j
                            for kc in range(KC):
                                nc.tensor.matmul(
                                    po[64 * j : 64 * j + CH + 1, :],
                                    v_sb[:, kc, h, :],
                                    Es[:, kc, j, :],
                                    start=(kc == 0),
                                    stop=(kc == KC - 1),
                                )
                        # full-tile evacuation: partitions 33-63/97-127 are
                        # never written (dead data, never read downstream);
                        # one copy halves the PSUM-access overhead vs two.
                        nc.vector.tensor_copy(stg[:, pr, :], po)

                    prev = None
                    for pr in range(PR):
                        hg = pr // 2
                        Es = ee.tile([P, KC, 2, SEQ], bf16, tag="Es")
                        for kc in range(KC):
                            sp = psQ.tile([P, 2, SEQ], f32, tag="qk")
                            for j in range(2):
                                h2 = 2 * (pr % 2) + j
                                nc.tensor.matmul(
                                    sp[:, j, :],
                                    kT[
                                        CH * h2 : CH * (h2 + 1),
                                        hg,
                                        P * kc : P * (kc + 1),
                                    ],
                                    qT[CH * h2 : CH * (h2 + 1), hg, :],
                                    start=True,
                                    stop=True,
                                    tile_position=(CH * h2, 0),
                                )
                            nc.scalar.activation(
                                Es[:, kc, :, :],
                                sp,
                                Exp,
                                bias=bm_sb[:, kc, n : n + 1],
                            )
                        # bias_pair applied multiplicatively post-exp; all-bf16
                        # SBUF operands hit the DVE 2x_1p rate. Pair 3 runs on
                        # GPSIMD (slower but otherwise idle) to balance load.
                        if pr < 3:
                            nc.vector.tensor_mul(Es, Es, bpt_sb[:, pr])
                        else:
                            nc.gpsimd.tensor_mul(Es, Es, bpt_sb[:, pr])
                        if prev is not None:
                            emit_av(prev)
                        prev = (pr, Es)
                        if pr == 1 and tail_state is not None:
                            # row n-1's tail goes here so its denominator DMA
                            # chain has finished by the time the PE (in-order)
                            # reaches the outproj matmuls
                            emit_tail(tail_state)
                    emit_av(prev)

                    # repack the head-packed AV outputs into outproj layout:
                    # oTg_raw[32*(h%4)+c, h//4, q] = stg[64*(h%2)+c, h//2, q]
                    for j in range(2):
                        for x in range(2):
                            nc.sync.dma_start(
                                out=oTg_raw[64 * x + 32 * j : 64 * x + 32 * j + 32],
                                in_=stg[64 * j : 64 * j + 32].rearrange(
                                    "p (g x) q -> p x g q", g=HG
                                )[:, x],
                            )
                    # denominator rows (32/96 of each pair bank): SBUF
                    # partition-broadcast DMA is illegal, so bounce through
                    # DRAM. Reciprocal is free-dim bound (8 cyc/elem), so
                    # reshape the 8x512 denominators into [128, 32] for the
                    # DVE reciprocal (256 cyc), then broadcast the result to
                    # each head's 32-partition channel block.
                    dscr = drp.tile([2, PR, SEQ], f32, tag="dscr")
                    nc.sync.dma_start(
                        out=dscr,
                        in_=stg.rearrange(
                            "(a b p) pr q -> a b p pr q", a=2, b=2
                        )[:, 1, 0],
                    )
                    den128 = og.tile([P, CH], f32, tag="den128")
                    nc.sync.dma_start(
                        out=den128,
                        in_=dscr.rearrange("a r (s f) -> (a r s) f", f=CH),
                    )
                    rden = og.tile([P, CH], f32, tag="rden")
                    nc.vector.reciprocal(rden, den128)
                    rdscr = drp.tile([2, PR, SEQ], f32, tag="rdscr")
                    nc.sync.dma_start(
                        out=rdscr.rearrange("a r (s f) -> (a r s) f", f=CH),
                        in_=rden,
                    )
                    for h in range(H):
                        nc.sync.dma_start(
                            out=rbc[32 * (h % 4) : 32 * (h % 4) + 32, h // 4, :],
                            in_=rdscr[h % 2, h // 2 : h // 2 + 1, :].to_broadcast(
                                [32, SEQ]
                            ),
                        )
                    return (n, gth, oTg_raw, rbc)

                def emit_tail(state):
                    n, gth, oTg_raw, rbc = state
                    # finish sigmoid: g = 0.5*tanh + 0.5 (in place)
                    nc.gpsimd.tensor_scalar(gth, gth, 0.5, 0.5, MULT, ADD)
                    # gr = g * (1/den); rbc holds the broadcast reciprocal
                    gr = og.tile([P, HG, SEQ], f32, tag="gr")
                    nc.gpsimd.tensor_mul(gr, gth, rbc)
                    oTg = oo.tile([P, HG, SEQ], bf16, tag="oTg")
                    nc.gpsimd.tensor_mul(oTg, oTg_raw, gr)
                    osb = ou.tile([P, QC, C], f32, tag="osb")
                    for qc in range(QC):
                        pp = psP.tile([P, SEQ], f32, tag="psP")
                        for hc in range(HC):
                            nc.tensor.matmul(
                                pp[:, 0:C],
                                oTg[:, hc, P * qc : P * (qc + 1)],
                                wo_sb[:, hc, :],
                                start=(hc == 0),
                                stop=(hc == HC - 1),
                            )
                        nc.vector.scalar_tensor_tensor(
                            osb[:, qc, :], pp[:, 0:C], 0.0, bo_sb, BYPASS, ADD
                        )
                    nc.sync.dma_start(
                        out=out[n].rearrange("(qc p) c -> p qc c", p=P), in_=osb
                    )

                emit_dma_in(0)
                pending = None
                for n in range(NL):
                    front = emit_proj(n)
                    if n + 1 < NL:
                        emit_dma_in(n + 1)
                    pending = emit_pairs(n, *front, pending)
                emit_tail(pending)

    return nc


_NC_CACHE = None


def _get_nc():
    global _NC_CACHE
    if _NC_CACHE is None:
        _NC_CACHE = _build_nc()
    return _NC_CACHE


def _prepare_in_maps(q_x, k_x, v_x, bias_mask, bias_pair, wq, wk, wv, wg, bg, wo, bo):
    wq_s = np.ascontiguousarray(wq / math.sqrt(CH)).astype(BF)
    # EBP[p, pr, kc, j, q] = exp(bias_pair[h=2*pr+j, q, k=128*kc+p])
    ebp = np.exp(np.transpose(bias_pair[0, 0], (0, 2, 1)))  # [h, k, q]
    ebp = ebp.reshape(PR, 2, KC, P, SEQ)
    bpt_host = np.ascontiguousarray(ebp.transpose(3, 0, 2, 1, 4)).astype(BF)
    bgh = np.ascontiguousarray((bg / 2.0).reshape(HC, P).T, dtype=np.float32)
    bo_bc = np.ascontiguousarray(np.tile(bo[None, :], (P, 1)), dtype=np.float32)
    bm_all = np.asarray(bias_mask[0, :, 0, 0, :], dtype=np.float32)  # [64, 512]

    w_common = {
        "bpt": bpt_host,
        "wq": wq_s,
        "wk": np.ascontiguousarray(wk).astype(BF),
        "wv": np.ascontiguousarray(wv).astype(BF),
        "wg": np.ascontiguousarray(wg).astype(BF),
        "bgh": bgh,
        "wo": np.ascontiguousarray(wo).astype(BF),
        "bo_bc": bo_bc,
    }
    in_maps = []
    for c in range(N_CORES):
        ns = slice(NL * c, NL * (c + 1))
        bm_r = np.ascontiguousarray(
            bm_all[ns].reshape(NL, KC, P).transpose(2, 1, 0), dtype=np.float32
        )
        in_maps.append(
            {
                "qx": np.ascontiguousarray(q_x[0, ns].transpose(0, 2, 1)).astype(BF),
                "kx": np.ascontiguousarray(k_x[0, ns].transpose(0, 2, 1)).astype(BF),
                "vx": np.ascontiguousarray(v_x[0, ns].transpose(0, 2, 1)).astype(BF),
                "bm": bm_r,
                **w_common,
            }
        )
    return in_maps


def run(trace=False, **inputs):
    """Run the kernel; returns (output, BassKernelResults)."""
    args = {k: np.asarray(v) for k, v in inputs.items()}
    in_maps = _prepare_in_maps(
        args["q_x"], args["k_x"], args["v_x"], args["bias_mask"],
        args["bias_pair"], args["wq"], args["wk"], args["wv"], args["wg"],
        args["bg"], args["wo"], args["bo"],
    )
    nc = _get_nc()
    res = run_bass_kernel_spmd(nc, in_maps, list(range(N_CORES)), trace=trace)
    out = np.empty((1, NL * N_CORES, SEQ, C), dtype=np.float32)
    for c in range(N_CORES):
        out[0, NL * c : NL * (c + 1)] = res.results[c]["out"]
    return out, res


def kernel(**inputs):
    out, _ = run(trace=False, **inputs)
    return out


if __name__ == "__main__":
    rng = np.random.default_rng(0)
    demo = {
        "q_x": rng.standard_normal((1, 64, SEQ, C)).astype(np.float32),
        "k_x": rng.standard_normal((1, 64, SEQ, C)).astype(np.float32),
        "v_x": rng.standard_normal((1, 64, SEQ, C)).astype(np.float32),
        "bias_mask": rng.standard_normal((1, 64, 1, 1, SEQ)).astype(np.float32),
        "bias_pair": rng.standard_normal((1, 1, H, SEQ, SEQ)).astype(np.float32),
        "wq": (rng.standard_normal((C, HID)) / 16).astype(np.float32),
        "wk": (rng.standard_normal((C, HID)) / 16).astype(np.float32),
        "wv": (rng.standard_normal((C, HID)) / 16).astype(np.float32),
        "wg": (rng.standard_normal((C, HID)) * 0.02).astype(np.float32),
        "bg": np.ones((HID,), dtype=np.float32),
        "wo": (rng.standard_normal((HID, C)) * 0.02).astype(np.float32),
        "bo": np.zeros((C,), dtype=np.float32),
    }
    o = kernel(**demo)
    print("kernel ran, out shape", o.shape, "mean", float(np.abs(o).mean()))


# revision 25
# speedup vs baseline: 1.3847x; 1.3847x over previous
"""Trainium2 Bass kernel for AlphaFold-style gated MSA attention.

Reference computation (per batch b=1, per MSA row n of 64):
    q = (q_x @ wq) / sqrt(32);  k = k_x @ wk;  v = v_x @ wv      (heads: 8 x 32)
    a = softmax(q k^T + bias_mask[n,k] + bias_pair[h,q,k])
    o = (a @ v) * sigmoid(q_x @ wg + bg)
    out = o @ wo + bo

Distribution: data-parallel over the 64 MSA rows -> 8 rows per NeuronCore.

Design (v2, bf16):
  * All matmul operands bf16 (1 cyc/row on PE, same as f32r, but halves
    DMA/SBUF traffic and speeds weight loads). Logits/accumulations stay
    f32 in PSUM. Measured end-to-end rel err ~6e-3 vs the 2e-2 gate.
  * S^T layout ([k, q]) so bias_mask folds into the ACT exp as a
    per-partition bias. bias_pair is applied multiplicatively AFTER exp:
    host ships EBP = exp(bias_pair^T) in bf16; one DVE tensor_mul per
    head-pair runs at the 2x_1p rate (all-bf16), replacing the baseline's
    expensive PE-identity / GPSIMD bias paths.
  * exp runs on ACT in [128, 2, 512] chunks (2 heads x 1 key-chunk),
    writing bf16 E. No max-subtraction (logits are O(10), f32/bf16 safe).
  * AV packs 2 heads per PSUM bank (tile_position cols 0/64); the v tile
    carries a ones column so row 32/96 of each bank accumulates the
    softmax denominator for free.
  * The per-head [33, 512] outputs are staged to SBUF once per pair
    (GPSIMD copy), then repacked into outproj layout with 4 DMAs and the
    denominators broadcast with 8 tiny DMAs; 1/den via the fast DVE
    reciprocal approximation; gate = sigmoid via ACT tanh (same table as
    exp) + GPSIMD fixup.
  * Software pipelining: row n's tail (normalize/gate/outproj) is emitted
    during row n+1; AV of pair p is emitted during QK of pair p+1 so the
    PE never waits on the ACT exp pipeline.
"""

import math
import os
import sys

for _p in ("/opt/trn_rl_repo", "/root/.axon_site/_ro/trn_rl_repo"):
    if os.path.isdir(_p) and _p not in sys.path:
        sys.path.append(_p)

import ml_dtypes
import numpy as np

import bass_rust
import concourse.bass as bass
import concourse.mybir as mybir
import concourse.tile as tile
from concourse.bass_utils import run_bass_kernel_spmd
from concourse.tile import ScopedClock

f32 = mybir.dt.float32
bf16 = mybir.dt.bfloat16
BF = ml_dtypes.bfloat16

N_CORES = 8
NL = 8        # MSA rows per core (64 / 8)
SEQ = 512     # q and k sequence length
C = 256       # channel dim of q_x/k_x/v_x and the output
HID = 256     # heads * c_hidden
H = 8         # heads
CH = 32       # c_hidden per head
P = 128
CC = C // P   # 2 contraction chunks for projections
HC = HID // P  # 2 hidden chunks
KC = SEQ // P  # 4 key chunks
QC = SEQ // P  # 4 query chunks
HG = 2        # head groups of 4
PR = 4        # head pairs


class _TileContextSplitWaits(tile.TileContext):
    """This container's walrus supports ONE sync-wait per instruction (the
    TRN2 EVENTS struct has a single wait slot and this build refuses to
    expand multi-wait instructions). Tile attaches several waits to one
    instruction; split the extras onto same-engine NOPs emitted just before
    it — the engine queue is in-order, so this is semantically identical."""

    def _add_instruction(self, inst):
        si = inst.sync_info
        if (
            si is not None
            and len(si.on_wait) > 1
            and inst.engine != mybir.EngineType.Unassigned
        ):
            waits = list(si.on_wait)
            for w in waits[:-1]:
                nop = mybir.InstNoOp(
                    name=self.nc.get_next_instruction_name(),
                    sync_info=mybir.SyncInfo(on_wait=[w], on_update=[]),
                    bass_nofuse=True,
                    engine=inst.engine,
                )
                super()._add_instruction(nop)
            inst.sync_info = mybir.SyncInfo(
                on_wait=waits[-1:], on_update=list(si.on_update)
            )
        super()._add_instruction(inst)

    def _drain_and_barrier(self, tick_clock, wait_clock):
        nc = self.nc
        drain_inst = nc.sync.drain()
        wait_clock.add_sem_waits(
            drain_inst.ins, ScopedClock({None: tick_clock.global_clock})
        )
        si = drain_inst.ins.sync_info
        if si is not None and len(si.on_wait) > 1:
            waits = list(si.on_wait)
            updates = list(si.on_update)
            drain_inst.ins.sync_info = bass_rust.SyncInfo(
                on_wait=waits[:1], on_update=[]
            )
            for i, w in enumerate(waits[1:]):
                upd = updates if i == len(waits) - 2 else []
                nop = nc.sync.nop()
                nop.ins.sync_info = bass_rust.SyncInfo(on_wait=[w], on_update=upd)
        nc.all_engine_barrier()
        assert self.sems is not None
        popped = nc._tile_sem_poison_stack.pop()
        assert popped is self._sem_poison
        nc.clear_and_free_semaphores(list(self.sems.allocated().values()))
        nc.all_engine_barrier()


def _build_nc():
    nc = bass.Bass(
        "TRN2", target_bir_lowering=False, debug=False, num_devices=N_CORES
    )
    qx = nc.dram_tensor("qx", [NL, C, SEQ], bf16, kind="ExternalInput").ap()
    kx = nc.dram_tensor("kx", [NL, C, SEQ], bf16, kind="ExternalInput").ap()
    vx = nc.dram_tensor("vx", [NL, C, SEQ], bf16, kind="ExternalInput").ap()
    bpt = nc.dram_tensor(
        "bpt", [P, HG, KC, 4, SEQ], bf16, kind="ExternalInput"
    ).ap()
    bm = nc.dram_tensor("bm", [P, KC, NL], f32, kind="ExternalInput").ap()
    wq = nc.dram_tensor("wq", [C, HID], bf16, kind="ExternalInput").ap()
    wk = nc.dram_tensor("wk", [C, HID], bf16, kind="ExternalInput").ap()
    wv = nc.dram_tensor("wv", [C, HID], bf16, kind="ExternalInput").ap()
    wg = nc.dram_tensor("wg", [C, HID], bf16, kind="ExternalInput").ap()
    bgh = nc.dram_tensor("bgh", [P, HC], f32, kind="ExternalInput").ap()
    wo = nc.dram_tensor("wo", [HID, C], bf16, kind="ExternalInput").ap()
    bo_bc = nc.dram_tensor("bo_bc", [P, C], f32, kind="ExternalInput").ap()
    out = nc.dram_tensor("out", [NL, SEQ, C], f32, kind="ExternalOutput").ap()

    Exp = mybir.ActivationFunctionType.Exp
    Tanh = mybir.ActivationFunctionType.Tanh
    Copy = mybir.ActivationFunctionType.Copy
    MULT = mybir.AluOpType.mult
    ADD = mybir.AluOpType.add
    BYPASS = mybir.AluOpType.bypass

    with _TileContextSplitWaits(nc) as tc:
        with (
            tc.tile_pool(name="const", bufs=1) as const,
        ):
            # --- constants ---------------------------------------------------
            w_sbs = {}
            for name, w_ap in (("wq", wq), ("wk", wk), ("wv", wv), ("wg", wg)):
                w_sbs[name] = const.tile(
                    [P, CC, HID], bf16, tag=f"w_{name}", name=f"w_{name}"
                )
                nc.sync.dma_start(
                    out=w_sbs[name],
                    in_=w_ap.rearrange("(cc p) h -> p cc h", p=P),
                )
            wo_sb = const.tile([P, HC, C], bf16, tag="w_wo")
            nc.sync.dma_start(
                out=wo_sb, in_=wo.rearrange("(hc p) c -> p hc c", p=P)
            )
            bm_sb = const.tile([P, KC, NL], f32, tag="bm")
            nc.sync.dma_start(out=bm_sb, in_=bm)
            bgh_sb = const.tile([P, HC], f32, tag="bgh")
            nc.sync.dma_start(out=bgh_sb, in_=bgh)
            bo_sb = const.tile([P, C], f32, tag="bo")
            nc.sync.dma_start(out=bo_sb, in_=bo_bc)
            # exp(bias_pair^T) [p, g, kc, j, q]; DMA'd after the first row's
            # inputs (see below) so row 0's projections start promptly
            bpt_sb = const.tile([P, HG, KC, 4, SEQ], bf16, tag="bpt")

            # --- main loop ---------------------------------------------------
            with (
                tc.tile_pool(name="xt", bufs=2) as xt,
                tc.tile_pool(name="drp", bufs=2, space="DRAM") as drp,
                tc.tile_pool(name="qk", bufs=2) as qk,
                tc.tile_pool(name="gh", bufs=2) as gh,
                tc.tile_pool(name="vv", bufs=2) as vv,
                tc.tile_pool(name="ee", bufs=2) as ee,
                tc.tile_pool(name="st", bufs=2) as st,
                tc.tile_pool(name="og", bufs=2) as og,
                tc.tile_pool(name="oo", bufs=2) as oo,
                tc.tile_pool(name="ou", bufs=2) as ou,
                tc.tile_pool(name="psP", bufs=2, space="PSUM") as psP,
                tc.tile_pool(name="psQ", bufs=1, space="PSUM") as psQ,
                tc.tile_pool(name="psO", bufs=2, space="PSUM") as psO,
            ):
                xts = {}

                def emit_dma_in(n):
                    t = {}
                    for name, src in (("q", qx), ("k", kx), ("v", vx)):
                        xT = xt.tile([P, CC, SEQ], bf16, tag=f"xt_{name}")
                        nc.sync.dma_start(
                            out=xT,
                            in_=src[n].rearrange("(cc p) s -> p cc s", p=P),
                        )
                        t[name] = xT
                    xts[n] = t

                def emit_proj(n):
                    xT = xts.pop(n)
                    qT = qk.tile([P, HG, SEQ], bf16, tag="qT")
                    kT = qk.tile([P, HG, SEQ], bf16, tag="kT")
                    for dst, wname, src in (
                        (qT, "wq", xT["q"]),
                        (kT, "wk", xT["k"]),
                    ):
                        for hc in range(HC):
                            pp = psP.tile([P, SEQ], f32, tag="psP")
                            for cc in range(CC):
                                nc.tensor.matmul(
                                    pp,
                                    w_sbs[wname][:, cc, P * hc : P * (hc + 1)],
                                    src[:, cc, :],
                                    start=(cc == 0),
                                    stop=(cc == CC - 1),
                                )
                            # ACT owns these evacuations (copy shares the exp
                            # table); DVE is saturated by the EBP multiplies
                            nc.scalar.activation(dst[:, hc, :], pp, Copy)

                    gth = gh.tile([P, HC, SEQ], f32, tag="gth")
                    for hc in range(HC):
                        pp = psP.tile([P, SEQ], f32, tag="psP")
                        for cc in range(CC):
                            nc.tensor.matmul(
                                pp,
                                w_sbs["wg"][:, cc, P * hc : P * (hc + 1)],
                                xT["q"][:, cc, :],
                                start=(cc == 0),
                                stop=(cc == CC - 1),
                            )
                        # sigmoid(x + bg) = 0.5*tanh((x + bg)/2) + 0.5;
                        # the 0.5t+0.5 fixup runs on GPSIMD in the tail.
                        nc.scalar.activation(
                            gth[:, hc, :],
                            pp,
                            Tanh,
                            bias=bgh_sb[:, hc : hc + 1],
                            scale=0.5,
                        )

                    v_sb = vv.tile([P, KC, H, CH + 1], bf16, tag="v")
                    nc.gpsimd.memset(v_sb[:, :, :, CH : CH + 1], 1.0)
                    for rc in range(KC):
                        pp = psP.tile([P, SEQ], f32, tag="psP")
                        for cc in range(CC):
                            nc.tensor.matmul(
                                pp[:, 0:HID],
                                xT["v"][:, cc, P * rc : P * (rc + 1)],
                                w_sbs["wv"][:, cc, :],
                                start=(cc == 0),
                                stop=(cc == CC - 1),
                            )
                        nc.vector.tensor_copy(
                            v_sb[:, rc, :, 0:CH],
                            pp[:, 0:HID].rearrange("p (h c) -> p h c", h=H),
                        )
                    return qT, kT, gth, v_sb

                # One attention "group" = 4 heads (= one qT/kT chunk). The AV
                # matmuls of group G are emitted while the NEXT group's QK/exp
                # pipeline runs, so the PE stays dense. `av_lag` carries
                # (row_ctx, group, Es) across the one-group delay; row_ctx
                # carries the per-row tiles.
                av_lag = [None]

                def emit_av_quarter(item, kc):
                    ctx, g, Es = item
                    if kc == 0:
                        ctx["po"][g] = [
                            psO.tile([P, SEQ], f32, tag="po", name=f"po{ab}")
                            for ab in range(2)
                        ]
                    for ab in range(2):
                        for j in range(2):
                            h = 4 * g + 2 * ab + j
                            nc.tensor.matmul(
                                ctx["po"][g][ab][64 * j : 64 * j + CH + 1, :],
                                ctx["v_sb"][:, kc, h, :],
                                Es[:, kc, 2 * ab + j, :],
                                start=(kc == 0),
                                stop=(kc == KC - 1),
                            )
                    if kc == KC - 1:
                        for ab in range(2):
                            # full-tile evacuation: partitions 33-63/97-127
                            # are dead data, never read downstream; one copy
                            # halves the PSUM-access overhead vs two.
                            nc.vector.tensor_copy(
                                ctx["stg"][:, 2 * g + ab, :], ctx["po"][g][ab]
                            )
                        if g == HG - 1:
                            emit_den_repack(ctx)

                def emit_den_repack(ctx):
                    stg, oTg_raw, rbc = ctx["stg"], ctx["oTg_raw"], ctx["rbc"]
                    # repack the head-packed AV outputs into outproj layout:
                    # oTg_raw[32*(h%4)+c, h//4, q] = stg[64*(h%2)+c, h//2, q]
                    for j in range(2):
                        for x in range(2):
                            nc.sync.dma_start(
                                out=oTg_raw[
                                    64 * x + 32 * j : 64 * x + 32 * j + 32
                                ],
                                in_=stg[64 * j : 64 * j + 32].rearrange(
                                    "p (g x) q -> p x g q", g=HG
                                )[:, x],
                            )
                    # denominator rows (32/96 of each bank): SBUF
                    # partition-broadcast DMA is illegal, so bounce through
                    # DRAM. Reciprocal is free-dim bound (8 cyc/elem), so
                    # reshape the 8x512 denominators into [128, 32] for the
                    # DVE reciprocal (256 cyc), then broadcast the result to
                    # each head's 32-partition channel block.
                    dscr = drp.tile([2, PR, SEQ], f32, tag="dscr")
                    nc.sync.dma_start(
                        out=dscr,
                        in_=stg.rearrange(
                            "(a b p) pr q -> a b p pr q", a=2, b=2
                        )[:, 1, 0],
                    )
                    den128 = og.tile([P, CH], f32, tag="den128")
                    nc.sync.dma_start(
                        out=den128,
                        in_=dscr.rearrange("a r (s f) -> (a r s) f", f=CH),
                    )
                    rden = og.tile([P, CH], f32, tag="rden")
                    nc.vector.reciprocal(rden, den128)
                    rdscr = drp.tile([2, PR, SEQ], f32, tag="rdscr")
                    nc.sync.dma_start(
                        out=rdscr.rearrange("a r (s f) -> (a r s) f", f=CH),
                        in_=rden,
                    )
                    for h in range(H):
                        nc.sync.dma_start(
                            out=rbc[32 * (h % 4) : 32 * (h % 4) + 32, h // 4, :],
                            in_=rdscr[
                                h % 2, h // 2 : h // 2 + 1, :
                            ].to_broadcast([32, SEQ]),
                        )

                def emit_group(ctx, g):
                    n, qT, kT = ctx["n"], ctx["qT"], ctx["kT"]
                    Es = ee.tile([P, KC, 4, SEQ], bf16, tag="Es")
                    for kc in range(KC):
                        sp = psQ.tile([P, 4, SEQ], f32, tag="qk")
                        for j in range(4):
                            nc.tensor.matmul(
                                sp[:, j, :],
                                kT[
                                    CH * j : CH * (j + 1),
                                    g,
                                    P * kc : P * (kc + 1),
                                ],
                                qT[CH * j : CH * (j + 1), g, :],
                                start=True,
                                stop=True,
                                tile_position=(CH * j, 0),
                            )
                        nc.scalar.activation(
                            Es[:, kc, :, :],
                            sp,
                            Exp,
                            bias=bm_sb[:, kc, n : n + 1],
                        )
                        # bias_pair applied multiplicatively post-exp; all-bf16
                        # SBUF operands hit the DVE 2x_1p rate. Split per kc so
                        # the lagged AV of this group never waits long.
                        nc.vector.tensor_mul(
                            Es[:, kc], Es[:, kc], bpt_sb[:, g, kc]
                        )
                        if av_lag[0] is not None:
                            emit_av_quarter(av_lag[0], kc)
                    av_lag[0] = (ctx, g, Es)

                def emit_tail(ctx):
                    n, gth = ctx["n"], ctx["gth"]
                    oTg_raw, rbc = ctx["oTg_raw"], ctx["rbc"]
                    # finish sigmoid: g = 0.5*tanh + 0.5 (in place)
                    nc.gpsimd.tensor_scalar(gth, gth, 0.5, 0.5, MULT, ADD)
                    # gr = g * (1/den); rbc holds the broadcast reciprocal
                    gr = og.tile([P, HG, SEQ], f32, tag="gr")
                    nc.gpsimd.tensor_mul(gr, gth, rbc)
                    oTg = oo.tile([P, HG, SEQ], bf16, tag="oTg")
                    nc.gpsimd.tensor_mul(oTg, oTg_raw, gr)
                    osb = ou.tile([P, QC, C], f32, tag="osb")
                    for qc in range(QC):
                        pp = psP.tile([P, SEQ], f32, tag="psP")
                        for hc in range(HC):
                            nc.tensor.matmul(
                                pp[:, 0:C],
                                oTg[:, hc, P * qc : P * (qc + 1)],
                                wo_sb[:, hc, :],
                                start=(hc == 0),
                                stop=(hc == HC - 1),
                            )
                        nc.vector.scalar_tensor_tensor(
                            osb[:, qc, :], pp[:, 0:C], 0.0, bo_sb, BYPASS, ADD
                        )
                    nc.sync.dma_start(
                        out=out[n].rearrange("(qc p) c -> p qc c", p=P), in_=osb
                    )

                emit_dma_in(0)
                # bulk bias table goes in after row 0's inputs, split per
                # group so group 0's slice lands before the first QK drains
                for g in range(HG):
                    nc.sync.dma_start(out=bpt_sb[:, g], in_=bpt[:, g])

                pending = None
                for n in range(NL):
                    qT, kT, gth, v_sb = emit_proj(n)
                    if n + 1 < NL:
                        emit_dma_in(n + 1)
                    ctx = {
                        "n": n, "qT": qT, "kT": kT, "gth": gth, "v_sb": v_sb,
                        "stg": st.tile([P, PR, SEQ], f32, tag="stg", name="stg"),
                        "oTg_raw": og.tile(
                            [P, HG, SEQ], f32, tag="oTg_raw", name="oTg_raw"
                        ),
                        "rbc": og.tile([P, HG, SEQ], f32, tag="rbc", name="rbc"),
                        "po": {},
                    }
                    emit_group(ctx, 0)
                    emit_group(ctx, 1)
                    if pending is not None:
                        emit_tail(pending)
                    pending = ctx
                # drain: AV of the last group, then the last tail
                for kc in range(KC):
                    emit_av_quarter(av_lag[0], kc)
                emit_tail(pending)

    return nc


_NC_CACHE = None


def _get_nc():
    global _NC_CACHE
    if _NC_CACHE is None:
        _NC_CACHE = _build_nc()
    return _NC_CACHE


def _prepare_in_maps(q_x, k_x, v_x, bias_mask, bias_pair, wq, wk, wv, wg, bg, wo, bo):
    wq_s = np.ascontiguousarray(wq / math.sqrt(CH)).astype(BF)
    # EBP[p, g, kc, j, q] = exp(bias_pair[h=4*g+j, q, k=128*kc+p])
    ebp = np.exp(np.transpose(bias_pair[0, 0], (0, 2, 1)))  # [h, k, q]
    ebp = ebp.reshape(HG, 4, KC, P, SEQ)
    bpt_host = np.ascontiguousarray(ebp.transpose(3, 0, 2, 1, 4)).astype(BF)
    bgh = np.ascontiguousarray((bg / 2.0).reshape(HC, P).T, dtype=np.float32)
    bo_bc = np.ascontiguousarray(np.tile(bo[None, :], (P, 1)), dtype=np.float32)
    bm_all = np.asarray(bias_mask[0, :, 0, 0, :], dtype=np.float32)  # [64, 512]

    w_common = {
        "bpt": bpt_host,
        "wq": wq_s,
        "wk": np.ascontiguousarray(wk).astype(BF),
        "wv": np.ascontiguousarray(wv).astype(BF),
        "wg": np.ascontiguousarray(wg).astype(BF),
        "bgh": bgh,
        "wo": np.ascontiguousarray(wo).astype(BF),
        "bo_bc": bo_bc,
    }
    in_maps = []
    for c in range(N_CORES):
        ns = slice(NL * c, NL * (c + 1))
        bm_r = np.ascontiguousarray(
            bm_all[ns].reshape(NL, KC, P).transpose(2, 1, 0), dtype=np.float32
        )
        in_maps.append(
            {
                "qx": np.ascontiguousarray(q_x[0, ns].transpose(0, 2, 1)).astype(BF),
                "kx": np.ascontiguousarray(k_x[0, ns].transpose(0, 2, 1)).astype(BF),
                "vx": np.ascontiguousarray(v_x[0, ns].transpose(0, 2, 1)).astype(BF),
                "bm": bm_r,
                **w_common,
            }
        )
    return in_maps


def run(trace=False, **inputs):
    """Run the kernel; returns (output, BassKernelResults)."""
    args = {k: np.asarray(v) for k, v in inputs.items()}
    in_maps = _prepare_in_maps(
        args["q_x"], args["k_x"], args["v_x"], args["bias_mask"],
        args["bias_pair"], args["wq"], args["wk"], args["wv"], args["wg"],
        args["bg"], args["wo"], args["bo"],
    )
    nc = _get_nc()
    res = run_bass_kernel_spmd(nc, in_maps, list(range(N_CORES)), trace=trace)
    out = np.empty((1, NL * N_CORES, SEQ, C), dtype=np.float32)
    for c in range(N_CORES):
        out[0, NL * c : NL * (c + 1)] = res.results[c]["out"]
    return out, res


def kernel(**inputs):
    out, _ = run(trace=False, **inputs)
    return out


if __name__ == "__main__":
    rng = np.random.default_rng(0)
    demo = {
        "q_x": rng.standard_normal((1, 64, SEQ, C)).astype(np.float32),
        "k_x": rng.standard_normal((1, 64, SEQ, C)).astype(np.float32),
        "v_x": rng.standard_normal((1, 64, SEQ, C)).astype(np.float32),
        "bias_mask": rng.standard_normal((1, 64, 1, 1, SEQ)).astype(np.float32),
        "bias_pair": rng.standard_normal((1, 1, H, SEQ, SEQ)).astype(np.float32),
        "wq": (rng.standard_normal((C, HID)) / 16).astype(np.float32),
        "wk": (rng.standard_normal((C, HID)) / 16).astype(np.float32),
        "wv": (rng.standard_normal((C, HID)) / 16).astype(np.float32),
        "wg": (rng.standard_normal((C, HID)) * 0.02).astype(np.float32),
        "bg": np.ones((HID,), dtype=np.float32),
        "wo": (rng.standard_normal((HID, C)) * 0.02).astype(np.float32),
        "bo": np.zeros((C,), dtype=np.float32),
    }
    o = kernel(**demo)
    print("kernel ran, out shape", o.shape, "mean", float(np.abs(o).mean()))


# revision 57
# speedup vs baseline: 1.4932x; 1.0783x over previous
"""Trainium2 Bass kernel for AlphaFold-style gated MSA attention.

Reference computation (per batch b=1, per MSA row n of 64):
    q = (q_x @ wq) / sqrt(32);  k = k_x @ wk;  v = v_x @ wv      (heads: 8 x 32)
    a = softmax(q k^T + bias_mask[n,k] + bias_pair[h,q,k])
    o = (a @ v) * sigmoid(q_x @ wg + bg)
    out = o @ wo + bo

Distribution: data-parallel over the 64 MSA rows -> 8 rows per NeuronCore.

Design (v3, bf16, deep software pipelining). Measured 286us vs the
379us baseline on 8 NeuronCores; rel err ~6e-3 vs the 2e-2 gate.
  * All matmul operands bf16 (1 cyc/row on the PE, same rate as f32r,
    but half the DMA/SBUF traffic and faster weight loads);
    logits/accumulations stay f32 in PSUM.
  * S^T layout ([k, q]) so bias_mask folds into the ACT exp as a
    per-partition bias. bias_pair is applied multiplicatively AFTER the
    exp: the host ships EBP = exp(bias_pair^T) in bf16, and a DVE
    tensor_mul at the 2x_1p rate (all-bf16; pair 3 of each row on the
    otherwise-idle GPSIMD) replaces the baseline's expensive
    PE-identity / GPSIMD additive-bias paths.
  * exp on ACT in [128, 2, 512] chunks out of double-buffered 2-bank QK
    PSUM tiles, so QK(kc+2) overlaps exp(kc). No max-subtraction
    (logits are O(10), safe in f32/bf16).
  * AV packs 2 heads per PSUM bank (tile_position cols 0/64); the v
    tile carries a ones column so rows 32/96 of each bank accumulate
    the softmax denominator for free.
  * Pipeline schedule (phases overlap across rows):
      - AV of pair p is emitted two pairs late, interleaved per
        key-chunk ahead of the QK matmuls, so the PE stays dense
        through the ACT-paced exp stream;
      - row n+1's projections are a generator whose ~10 PSUM-group
        steps are spread across row n's 16 attention iterations;
      - the tail (denominator DRAM-bounce broadcast + [128, 32]-shaped
        DVE reciprocal, sigmoid fixup, gate multiply, output
        projection) runs two rows behind, so its ~10us latency chain
        never blocks the in-order engine queues.
  * PSUM budget (8 banks): 2x2 QK + 2x1 AV + 2x1 proj/outproj.
"""

import math
import os
import sys

for _p in ("/opt/trn_rl_repo", "/root/.axon_site/_ro/trn_rl_repo"):
    if os.path.isdir(_p) and _p not in sys.path:
        sys.path.append(_p)

import ml_dtypes
import numpy as np

import bass_rust
import concourse.bass as bass
import concourse.mybir as mybir
import concourse.tile as tile
from concourse.bass_utils import run_bass_kernel_spmd
from concourse.tile import ScopedClock

f32 = mybir.dt.float32
bf16 = mybir.dt.bfloat16
BF = ml_dtypes.bfloat16

N_CORES = 8
NL = 8        # MSA rows per core (64 / 8)
SEQ = 512     # q and k sequence length
C = 256       # channel dim of q_x/k_x/v_x and the output
HID = 256     # heads * c_hidden
H = 8         # heads
CH = 32       # c_hidden per head
P = 128
CC = C // P   # 2 contraction chunks for projections
HC = HID // P  # 2 hidden chunks
KC = SEQ // P  # 4 key chunks
QC = SEQ // P  # 4 query chunks
HG = 2        # head groups of 4
PR = 4        # head pairs


class _TileContextSplitWaits(tile.TileContext):
    """This container's walrus supports ONE sync-wait per instruction (the
    TRN2 EVENTS struct has a single wait slot and this build refuses to
    expand multi-wait instructions). Tile attaches several waits to one
    instruction; split the extras onto same-engine NOPs emitted just before
    it — the engine queue is in-order, so this is semantically identical."""

    def _add_instruction(self, inst):
        si = inst.sync_info
        if (
            si is not None
            and len(si.on_wait) > 1
            and inst.engine != mybir.EngineType.Unassigned
        ):
            waits = list(si.on_wait)
            for w in waits[:-1]:
                nop = mybir.InstNoOp(
                    name=self.nc.get_next_instruction_name(),
                    sync_info=mybir.SyncInfo(on_wait=[w], on_update=[]),
                    bass_nofuse=True,
                    engine=inst.engine,
                )
                super()._add_instruction(nop)
            inst.sync_info = mybir.SyncInfo(
                on_wait=waits[-1:], on_update=list(si.on_update)
            )
        super()._add_instruction(inst)

    def _drain_and_barrier(self, tick_clock, wait_clock):
        nc = self.nc
        drain_inst = nc.sync.drain()
        wait_clock.add_sem_waits(
            drain_inst.ins, ScopedClock({None: tick_clock.global_clock})
        )
        si = drain_inst.ins.sync_info
        if si is not None and len(si.on_wait) > 1:
            waits = list(si.on_wait)
            updates = list(si.on_update)
            drain_inst.ins.sync_info = bass_rust.SyncInfo(
                on_wait=waits[:1], on_update=[]
            )
            for i, w in enumerate(waits[1:]):
                upd = updates if i == len(waits) - 2 else []
                nop = nc.sync.nop()
                nop.ins.sync_info = bass_rust.SyncInfo(on_wait=[w], on_update=upd)
        nc.all_engine_barrier()
        assert self.sems is not None
        popped = nc._tile_sem_poison_stack.pop()
        assert popped is self._sem_poison
        nc.clear_and_free_semaphores(list(self.sems.allocated().values()))
        nc.all_engine_barrier()


def _build_nc():
    nc = bass.Bass(
        "TRN2", target_bir_lowering=False, debug=False, num_devices=N_CORES
    )
    qx = nc.dram_tensor("qx", [NL, C, SEQ], bf16, kind="ExternalInput").ap()
    kx = nc.dram_tensor("kx", [NL, C, SEQ], bf16, kind="ExternalInput").ap()
    vx = nc.dram_tensor("vx", [NL, C, SEQ], bf16, kind="ExternalInput").ap()
    bpt = nc.dram_tensor(
        "bpt", [P, PR, KC, 2, SEQ], bf16, kind="ExternalInput"
    ).ap()
    bm = nc.dram_tensor("bm", [P, KC, NL], f32, kind="ExternalInput").ap()
    wq = nc.dram_tensor("wq", [C, HID], bf16, kind="ExternalInput").ap()
    wk = nc.dram_tensor("wk", [C, HID], bf16, kind="ExternalInput").ap()
    wv = nc.dram_tensor("wv", [C, HID], bf16, kind="ExternalInput").ap()
    wg = nc.dram_tensor("wg", [C, HID], bf16, kind="ExternalInput").ap()
    bgh = nc.dram_tensor("bgh", [P, HC], f32, kind="ExternalInput").ap()
    wo = nc.dram_tensor("wo", [HID, C], bf16, kind="ExternalInput").ap()
    bo_bc = nc.dram_tensor("bo_bc", [P, C], f32, kind="ExternalInput").ap()
    out = nc.dram_tensor("out", [NL, SEQ, C], f32, kind="ExternalOutput").ap()

    Exp = mybir.ActivationFunctionType.Exp
    Tanh = mybir.ActivationFunctionType.Tanh
    Copy = mybir.ActivationFunctionType.Copy
    MULT = mybir.AluOpType.mult
    ADD = mybir.AluOpType.add
    BYPASS = mybir.AluOpType.bypass

    with _TileContextSplitWaits(nc) as tc:
        with (
            tc.tile_pool(name="const", bufs=1) as const,
        ):
            # --- constants ---------------------------------------------------
            w_sbs = {}
            for name, w_ap in (("wq", wq), ("wk", wk), ("wv", wv), ("wg", wg)):
                w_sbs[name] = const.tile(
                    [P, CC, HID], bf16, tag=f"w_{name}", name=f"w_{name}"
                )
                nc.sync.dma_start(
                    out=w_sbs[name],
                    in_=w_ap.rearrange("(cc p) h -> p cc h", p=P),
                )
            wo_sb = const.tile([P, HC, C], bf16, tag="w_wo")
            nc.sync.dma_start(
                out=wo_sb, in_=wo.rearrange("(hc p) c -> p hc c", p=P)
            )
            bm_sb = const.tile([P, KC, NL], f32, tag="bm")
            nc.sync.dma_start(out=bm_sb, in_=bm)
            bgh_sb = const.tile([P, HC], f32, tag="bgh")
            nc.sync.dma_start(out=bgh_sb, in_=bgh)
            bo_sb = const.tile([P, C], f32, tag="bo")
            nc.sync.dma_start(out=bo_sb, in_=bo_bc)
            # exp(bias_pair^T) [p, pair, kc, j, q]; DMA'd after the first
            # row's inputs (see below) so row 0's projections start promptly
            bpt_sb = const.tile([P, PR, KC, 2, SEQ], bf16, tag="bpt")

            # --- main loop ---------------------------------------------------
            with (
                tc.tile_pool(name="xt", bufs=2) as xt,
                tc.tile_pool(name="drp", bufs=3, space="DRAM") as drp,
                tc.tile_pool(name="qk", bufs=2) as qk,
                tc.tile_pool(name="gh", bufs=4) as gh,
                tc.tile_pool(name="vv", bufs=3) as vv,
                tc.tile_pool(name="ee", bufs=4) as ee,
                tc.tile_pool(name="st", bufs=3) as st,
                tc.tile_pool(name="og", bufs=3) as og,
                tc.tile_pool(name="oo", bufs=3) as oo,
                tc.tile_pool(name="ou", bufs=3) as ou,
                tc.tile_pool(name="psP", bufs=2, space="PSUM") as psP,
                tc.tile_pool(name="psQ", bufs=2, space="PSUM") as psQ,
                tc.tile_pool(name="psO", bufs=2, space="PSUM") as psO,
            ):
                xts = {}

                def emit_dma_in(n):
                    t = {}
                    for name, src in (("q", qx), ("k", kx), ("v", vx)):
                        xT = xt.tile([P, CC, SEQ], bf16, tag=f"xt_{name}")
                        nc.sync.dma_start(
                            out=xT,
                            in_=src[n].rearrange("(cc p) s -> p cc s", p=P),
                        )
                        t[name] = xT
                    xts[n] = t

                ctxs = {}

                def proj_gen(n):
                    """Row-n projection emitted as a generator of PSUM-sized
                    work groups, so they can be interleaved into row n-1's
                    attention loop — keeps the PE dense through the ACT-paced
                    exp stream and the ACT busy through the projections."""
                    xT = xts.pop(n)
                    qT = qk.tile([P, HG, SEQ], bf16, tag="qT", name="qT")
                    kT = qk.tile([P, HG, SEQ], bf16, tag="kT", name="kT")
                    gth = gh.tile([P, HC, SEQ], f32, tag="gth", name="gth")
                    v_sb = vv.tile(
                        [P, KC, H, CH + 1], bf16, tag="v", name="v_sb"
                    )
                    ctxs[n] = {
                        "n": n, "qT": qT, "kT": kT, "gth": gth, "v_sb": v_sb,
                        "stg": st.tile([P, PR, SEQ], f32, tag="stg", name="stg"),
                        "oTg_raw": og.tile(
                            [P, HG, SEQ], f32, tag="oTg_raw", name="oTg_raw"
                        ),
                        "rbc": og.tile([P, HG, SEQ], f32, tag="rbc", name="rbc"),
                        "po": {},
                    }
                    yield
                    for dst, wname, src_ in (
                        (qT, "wq", xT["q"]),
                        (kT, "wk", xT["k"]),
                    ):
                        for hc in range(HC):
                            pp = psP.tile([P, SEQ], f32, tag="psP")
                            for cc in range(CC):
                                nc.tensor.matmul(
                                    pp,
                                    w_sbs[wname][:, cc, P * hc : P * (hc + 1)],
                                    src_[:, cc, :],
                                    start=(cc == 0),
                                    stop=(cc == CC - 1),
                                )
                            # chunk-0 evacuations (needed by the row's first
                            # QK) on ACT, chunk-1 on DVE
                            if hc == 0:
                                nc.scalar.activation(dst[:, hc, :], pp, Copy)
                            else:
                                nc.vector.tensor_copy(dst[:, hc, :], pp)
                            yield
                    for hc in range(HC):
                        pp = psP.tile([P, SEQ], f32, tag="psP")
                        for cc in range(CC):
                            nc.tensor.matmul(
                                pp,
                                w_sbs["wg"][:, cc, P * hc : P * (hc + 1)],
                                xT["q"][:, cc, :],
                                start=(cc == 0),
                                stop=(cc == CC - 1),
                            )
                        # sigmoid(x + bg) = 0.5*tanh((x + bg)/2) + 0.5;
                        # the 0.5t+0.5 fixup runs on GPSIMD in the tail.
                        nc.scalar.activation(
                            gth[:, hc, :],
                            pp,
                            Tanh,
                            bias=bgh_sb[:, hc : hc + 1],
                            scale=0.5,
                        )
                        yield
                    nc.gpsimd.memset(v_sb[:, :, :, CH : CH + 1], 1.0)
                    for rc in range(KC):
                        pp = psP.tile([P, SEQ], f32, tag="psP")
                        for cc in range(CC):
                            nc.tensor.matmul(
                                pp[:, 0:HID],
                                xT["v"][:, cc, P * rc : P * (rc + 1)],
                                w_sbs["wv"][:, cc, :],
                                start=(cc == 0),
                                stop=(cc == CC - 1),
                            )
                        nc.vector.tensor_copy(
                            v_sb[:, rc, :, 0:CH],
                            pp[:, 0:HID].rearrange("p (h c) -> p h c", h=H),
                        )
                        yield

                # The AV matmuls of pair p are emitted while pair p+2's QK/exp
                # pipeline runs (one 4-head group of lag), so the PE stays
                # dense and never waits on the ACT exp stream. `av_lag` is a
                # 2-deep FIFO of (row_ctx, pair, Es).
                av_lag = []

                def emit_av_eighth(item, kc, last=False):
                    ctx, pr, Es = item
                    if kc == 0:
                        ctx["po"][pr] = psO.tile(
                            [P, SEQ], f32, tag="po", name="po"
                        )
                    po = ctx["po"][pr]
                    for j in range(2):
                        h = 2 * pr + j
                        nc.tensor.matmul(
                            po[64 * j : 64 * j + CH + 1, :],
                            ctx["v_sb"][:, kc, h, :],
                            Es[:, kc, j, :],
                            start=(kc == 0),
                            stop=(kc == KC - 1),
                        )
                    if kc == KC - 1:
                        # full-tile evacuation: partitions 33-63/97-127 are
                        # dead data, never read downstream; one copy halves
                        # the PSUM-access overhead vs two. Pair 3's copy sits
                        # on the denominator critical path -> ACT (shorter
                        # queue at that point than DVE).
                        nc.vector.tensor_copy(ctx["stg"][:, pr, :], po)
                        if pr == PR - 1:
                            emit_den_repack(ctx, last=last)

                def emit_den_repack(ctx, last=False):
                    stg, oTg_raw, rbc = ctx["stg"], ctx["oTg_raw"], ctx["rbc"]
                    # repack the head-packed AV outputs into outproj layout:
                    # oTg_raw[32*(h%4)+c, h//4, q] = stg[64*(h%2)+c, h//2, q]
                    for j in range(2):
                        for x in range(2):
                            nc.sync.dma_start(
                                out=oTg_raw[
                                    64 * x + 32 * j : 64 * x + 32 * j + 32
                                ],
                                in_=stg[64 * j : 64 * j + 32].rearrange(
                                    "p (g x) q -> p x g q", g=HG
                                )[:, x],
                            )
                    # denominator rows (32/96 of each bank): SBUF
                    # partition-broadcast DMA is illegal, so bounce through
                    # DRAM. Reciprocal is free-dim bound (8 cyc/elem), so
                    # gather the 8x512 denominators straight into a [128, 32]
                    # layout for the DVE reciprocal (256 cyc), then broadcast
                    # the result to each head's 32-partition channel block.
                    # All issued from the Pool queue (near-zero dispatch cost,
                    # and the gr multiply naturally queues behind them).
                    dscr = drp.tile([2, PR, SEQ], f32, tag="dscr")
                    nc.sync.dma_start(
                        out=dscr,
                        in_=stg.rearrange(
                            "(a b p) pr q -> a b p pr q", a=2, b=2
                        )[:, 1, 0],
                    )
                    den128 = og.tile([P, CH], f32, tag="den128")
                    nc.sync.dma_start(
                        out=den128,
                        in_=dscr.rearrange("a r (s f) -> (a r s) f", f=CH),
                    )
                    rden = og.tile([P, CH], f32, tag="rden")
                    nc.vector.reciprocal(rden, den128)
                    rdscr = drp.tile([2, PR, SEQ], f32, tag="rdscr")
                    nc.sync.dma_start(
                        out=rdscr.rearrange("a r (s f) -> (a r s) f", f=CH),
                        in_=rden,
                    )
                    for h in range(H):
                        # during the drain the other queues are idle; spread
                        # the broadcasts so their issue time doesn't serialize
                        eng = (
                            (nc.sync, nc.scalar)[h % 2]
                            if last
                            else nc.sync
                        )
                        eng.dma_start(
                            out=rbc[32 * (h % 4) : 32 * (h % 4) + 32, h // 4, :],
                            in_=rdscr[
                                h % 2, h // 2 : h // 2 + 1, :
                            ].to_broadcast([32, SEQ]),
                        )

                def emit_pair(ctx, pr, drv=None):
                    n, qT, kT = ctx["n"], ctx["qT"], ctx["kT"]
                    hg = pr // 2
                    Es = ee.tile([P, KC, 2, SEQ], bf16, tag="Es")
                    for kc in range(KC):
                        if drv is not None:
                            drv(4 * pr + kc)
                        # the lagged AV first: its dependencies are two pairs
                        # old, so it never blocks the head of the PE queue
                        if len(av_lag) >= 2:
                            emit_av_eighth(av_lag[0], kc)
                        sp = psQ.tile([P, 2, SEQ], f32, tag="qk")
                        for j in range(2):
                            h2 = 2 * (pr % 2) + j
                            nc.tensor.matmul(
                                sp[:, j, :],
                                kT[
                                    CH * h2 : CH * (h2 + 1),
                                    hg,
                                    P * kc : P * (kc + 1),
                                ],
                                qT[CH * h2 : CH * (h2 + 1), hg, :],
                                start=True,
                                stop=True,
                                tile_position=(CH * h2, 0),
                            )
                        nc.scalar.activation(
                            Es[:, kc, :, :],
                            sp,
                            Exp,
                            bias=bm_sb[:, kc, n : n + 1],
                        )
                        # previous pair's bias_pair multiply, chunked per kc
                        # (DVE 2x_1p, all-bf16; pair 3 on the otherwise-idle
                        # GPSIMD) so this row's evacuations get DVE slots
                        if av_lag:
                            c2, p2_, E2 = av_lag[-1]
                            eng = nc.gpsimd if p2_ == PR - 1 else nc.vector
                            eng.tensor_mul(
                                E2[:, kc], E2[:, kc], bpt_sb[:, p2_, kc]
                            )
                    av_lag.append((ctx, pr, Es))
                    if len(av_lag) > 2:
                        av_lag.pop(0)

                def emit_tail_a(ctx, last=False):
                    # normalize+gate: emitted early in the next row so these
                    # ops sit near the FRONT of the DVE/GPSIMD queues — the
                    # outproj otherwise stalls the PE ~15us at each row end.
                    # The final row's tail runs during the drain when the DVE
                    # is idle — use it there for latency.
                    eng = nc.vector if last else nc.gpsimd
                    gth, oTg_raw, rbc = ctx["gth"], ctx["oTg_raw"], ctx["rbc"]
                    # finish sigmoid: g = 0.5*tanh + 0.5 (in place)
                    eng.tensor_scalar(gth, gth, 0.5, 0.5, MULT, ADD)
                    # gr = g * (1/den); rbc holds the broadcast reciprocal
                    gr = og.tile([P, HG, SEQ], f32, tag="gr")
                    eng.tensor_mul(gr, gth, rbc)
                    oTg = oo.tile([P, HG, SEQ], bf16, tag="oTg")
                    eng.tensor_mul(oTg, oTg_raw, gr)
                    ctx["oTg"] = oTg

                def emit_tail_b(ctx):
                    n, oTg = ctx["n"], ctx["oTg"]
                    osb = ou.tile([P, QC, C], f32, tag="osb")
                    for qc in range(QC):
                        pp = psP.tile([P, SEQ], f32, tag="psP")
                        for hc in range(HC):
                            nc.tensor.matmul(
                                pp[:, 0:C],
                                oTg[:, hc, P * qc : P * (qc + 1)],
                                wo_sb[:, hc, :],
                                start=(hc == 0),
                                stop=(hc == HC - 1),
                            )
                        nc.vector.scalar_tensor_tensor(
                            osb[:, qc, :], pp[:, 0:C], 0.0, bo_sb, BYPASS, ADD
                        )
                    nc.sync.dma_start(
                        out=out[n].rearrange("(qc p) c -> p qc c", p=P), in_=osb
                    )

                emit_dma_in(0)
                # bulk bias table goes in after row 0's inputs, split per
                # pair so pair 0's slice lands before the first exp drains
                for pr in range(PR):
                    nc.sync.dma_start(out=bpt_sb[:, pr], in_=bpt[:, pr])

                for _ in proj_gen(0):
                    pass  # row 0's projections run as a prologue
                pending = []
                gen_next = [None]

                def drive(it):
                    # spread the next row's ~10 projection groups across
                    # iterations 2..11 of this row's 16 attention iterations
                    if gen_next[0] is not None and 2 <= it < 12:
                        if next(gen_next[0], StopIteration) is StopIteration:
                            gen_next[0] = None

                for n in range(NL):
                    ctx = ctxs.pop(n)
                    if n + 1 < NL:
                        emit_dma_in(n + 1)
                        gen_next[0] = proj_gen(n + 1)
                        next(gen_next[0])  # allocate row n+1's tiles
                    emit_pair(ctx, 0, drive)
                    emit_pair(ctx, 1, drive)
                    if len(pending) >= 2:
                        emit_tail_a(pending[-2])
                    emit_pair(ctx, 2, drive)
                    if len(pending) >= 2:
                        emit_tail_b(pending.pop(0))
                    emit_pair(ctx, 3, drive)
                    while gen_next[0] is not None:
                        if next(gen_next[0], StopIteration) is StopIteration:
                            gen_next[0] = None
                    pending.append(ctx)
                # drain: EBP of the final pair, AV of the last two pairs
                # interleaved with the two pending tails
                c2, p2_, E2 = av_lag[-1]
                eng = nc.gpsimd if p2_ == PR - 1 else nc.vector
                for kc in range(KC):
                    eng.tensor_mul(E2[:, kc], E2[:, kc], bpt_sb[:, p2_, kc])
                items = list(av_lag)
                for kc in range(KC):
                    emit_av_eighth(items[0], kc)
                emit_tail_a(pending[0])
                for kc in range(KC):
                    emit_av_eighth(items[1], kc, last=True)
                emit_tail_b(pending[0])
                emit_tail_a(pending[1], last=True)
                emit_tail_b(pending[1])

    return nc
